# revision 10
# baseline (speedup 1.0000x reference)
"""MoE routing kernel for 8 Trainium2 NeuronCores.

Problem: nn_MoE_hard (moe_routing). Reference computes, per token (B=256,N=64):
  gate_scores = renorm(top2mask(softmax(x @ gate_W + gate_b)))      [B,N,E]
  out = local_top1_mask(x@lg_W) * sum_e gate[e]*(relu(x@W1[e]+b1[e])@W2[e]+b2[e])
Only ACTIVE_K=1 of the N=64 rows per batch entry survives the local mask, so
only B=256 tokens need the expert MLP.  Strategy:
  Phase A (data-parallel over batch, 32 batch entries/core):
    gating softmax + top-2 renorm for all tokens (the gate_scores output),
    local top-1 over N, gather of each batch entry's single active token.
  Phase B: AllGather of the 256 active tokens (+ their gate rows).
  Phase C (expert-parallel, 1 expert/core): dense 2-layer MLP over all 256
    active tokens for this core's expert, gate-weighted; ReduceScatter sums
    expert contributions and hands each core its own 32 batch rows, which are
    scattered into the (zero-initialized) output.
"""

import numpy as np

import concourse.bass as bass
import concourse.mybir as mybir
import concourse.tile as tile
from concourse import bacc
from concourse.bass import IndirectOffsetOnAxis
from concourse.bass_utils import run_bass_kernel_spmd
from concourse.masks import make_identity

F32 = mybir.dt.float32
F32R = mybir.dt.float32r
I32 = mybir.dt.int32

NCORES = 8
B, N, D = 256, 64, 1024
E, H = 8, 1024
BL = B // NCORES            # batch entries per core
TL = BL * N                 # tokens per core (2048)
G = TL // 128               # token groups of 128 (16)
J = E + 1                   # gate experts + local-gate column
A = B                       # total active tokens (ACTIVE_K=1 per batch entry)
KD = D // 128               # contraction chunks (8)
KH = H // 128               # hidden chunks (8)
MT = A // 128               # active-token 128-tiles (2)

AluOp = mybir.AluOpType
Act = mybir.ActivationFunctionType


def build():
    nc = bacc.Bacc("TRN2", num_devices=NCORES)

    xT = nc.dram_tensor("xT", [D, TL], F32, kind="ExternalInput")
    xrow = nc.dram_tensor("xrow", [TL, D], F32, kind="ExternalInput")
    gW = nc.dram_tensor("gW", [D, J], F32, kind="ExternalInput")
    gb = nc.dram_tensor("gb", [J, 1], F32, kind="ExternalInput")
    W1 = nc.dram_tensor("W1", [D, H], F32R, kind="ExternalInput")
    W2 = nc.dram_tensor("W2", [H, H], F32R, kind="ExternalInput")
    b1c = nc.dram_tensor("b1c", [128, KH], F32, kind="ExternalInput")
    b2c = nc.dram_tensor("b2c", [128, KH], F32, kind="ExternalInput")
    sel = nc.dram_tensor("sel", [E, 128], F32, kind="ExternalInput")

    gate_out = nc.dram_tensor("gate_out", [TL, E], F32, kind="ExternalOutput")
    out = nc.dram_tensor("out", [TL, H], F32, kind="ExternalOutput")

    loc_b = nc.dram_tensor("loc_b", [TL], F32)
    ag_in = nc.dram_tensor("ag_in", [BL, D + E], F32)
    ag_out = nc.dram_tensor("ag_out", [A, D + E], F32, addr_space="Shared")
    rs_in = nc.dram_tensor("rs_in", [A, H], F32)
    rs_out = nc.dram_tensor("rs_out", [BL, H], F32)

    groups = [list(range(NCORES))]

    with tile.TileContext(nc, num_cores=NCORES) as tc:
        with (
            tc.tile_pool(name="const", bufs=1) as cp,
            tc.tile_pool(name="big", bufs=1) as bp,
        ):
            # ---- critical-path constants first (DMA queue priority) ----
            gWsb = cp.tile([128, KD, J], F32, tag="gWsb")
            nc.sync.dma_start(
                gWsb[:], gW[:].rearrange("(k p) j -> p k j", p=128)
            )
            gbsb = cp.tile([J, 1], F32, tag="gbsb")
            nc.sync.dma_start(gbsb[:], gb[:])
            ident = cp.tile([128, 128], F32, tag="ident")
            make_identity(nc, ident[:])

            # ---- Phase A: gating for all local tokens ----
            # Pack the thin (M=9) gating matmul 4x across PE column groups:
            # k-chunk k accumulates into PSUM partitions [32*(k%4), 32*(k%4)+9).
            scoresT = bp.tile([J, TL], F32, tag="scoresT")
            scores3 = bp.tile([128, G, J], F32, tag="scores3")
            gatef = bp.tile([128, G, E], F32, tag="gatef")

            with (
                tc.tile_pool(name="xk", bufs=4) as xkp,
                tc.tile_pool(name="psA", bufs=1, space="PSUM") as psA,
                tc.tile_pool(name="psT", bufs=3, space="PSUM") as psT,
            ):
                sT_ps = psA.tile([128, TL], F32, tag="sT_ps")
                for k in range(KD):
                    xk = xkp.tile([128, TL], F32, tag="xk")
                    nc.sync.dma_start(xk[:], xT[k * 128 : (k + 1) * 128, :])
                    cg = 32 * (k % 4)
                    for n in range(TL // 512):
                        nc.tensor.matmul(
                            sT_ps[cg : cg + J, n * 512 : (n + 1) * 512],
                            lhsT=gWsb[:, k, :],
                            rhs=xk[:, n * 512 : (n + 1) * 512],
                            start=(k < 4),
                            stop=(k >= 4),
                            tile_position=(0, cg),
                        )
                # merge the 4 column groups + bias (PSUM -> SBUF); DVE may only
                # read one PSUM operand per op, so chain in-place adds
                nc.vector.tensor_scalar_add(scoresT[:], sT_ps[0:J, :], gbsb[:, 0:1])
                for cg in (32, 64, 96):
                    nc.vector.tensor_tensor(
                        out=scoresT[:], in0=scoresT[:], in1=sT_ps[cg : cg + J, :], op=AluOp.add
                    )

                # transpose to token-major [p, g, j]; token t = p*G + g
                sT_v = scoresT[:].rearrange("j (p g) -> j g p", g=G)
                for g in range(G):
                    tp = psT.tile([128, J], F32, tag="trA")
                    nc.tensor.transpose(tp[:], in_=sT_v[:, g, :], identity=ident[0:J, 0:J])
                    nc.vector.tensor_copy(scores3[:, g, :], tp[:])

            # ---- bulk weight loads (off critical path; scalar-engine ring) ----
            selsb = cp.tile([E, 128], F32, tag="selsb")
            nc.sync.dma_start(selsb[:], sel[:])
            b1csb = cp.tile([128, KH], F32, tag="b1csb")
            nc.sync.dma_start(b1csb[:], b1c[:])
            b2csb = cp.tile([128, KH], F32, tag="b2csb")
            nc.sync.dma_start(b2csb[:], b2c[:])
            W1sb = bp.tile([128, KD, H], F32R, tag="W1sb")
            nc.scalar.dma_start(W1sb[:], W1[:].rearrange("(k p) h -> p k h", p=128))
            W2sb = bp.tile([128, KH, H], F32R, tag="W2sb")
            nc.scalar.dma_start(W2sb[:], W2[:].rearrange("(k p) h -> p k h", p=128))

            # softmax over E (no max-subtract: logits are O(1)) + top-2 renorm
            expS = bp.tile([128, G, E], F32, tag="expS")
            probs = bp.tile([128, G, E], F32, tag="probs")
            m8 = bp.tile([128, G, E], F32, tag="m8")
            s16 = bp.tile([128, G], F32, tag="s16")
            r16 = bp.tile([128, G], F32, tag="r16")
            nc.scalar.activation(expS[:], scores3[:, :, 0:E], Act.Exp)
            nc.vector.reduce_sum(s16[:], expS[:], axis=mybir.AxisListType.X)
            nc.vector.reciprocal(r16[:], s16[:])
            bc = [128, G, E]
            nc.vector.tensor_tensor(
                out=probs[:], in0=expS[:], in1=r16[:, :, None].to_broadcast(bc), op=AluOp.mult
            )
            # top-1
            nc.vector.reduce_max(s16[:], probs[:], axis=mybir.AxisListType.X)
            nc.vector.tensor_tensor(
                out=m8[:], in0=probs[:], in1=s16[:, :, None].to_broadcast(bc), op=AluOp.is_ge
            )
            # knock out top-1 (probs <= 1, so subtracting the mask is enough)
            nc.vector.tensor_tensor(out=m8[:], in0=probs[:], in1=m8[:], op=AluOp.subtract)
            nc.vector.reduce_max(s16[:], m8[:], axis=mybir.AxisListType.X)
            # top-2 mask (>= second max)
            nc.vector.tensor_tensor(
                out=m8[:], in0=probs[:], in1=s16[:, :, None].to_broadcast(bc), op=AluOp.is_ge
            )
            nc.vector.tensor_tensor(out=gatef[:], in0=probs[:], in1=m8[:], op=AluOp.mult)
            nc.vector.reduce_sum(s16[:], gatef[:], axis=mybir.AxisListType.X)
            nc.vector.tensor_scalar_add(s16[:], s16[:], 1e-9)
            nc.vector.reciprocal(r16[:], s16[:])
            nc.vector.tensor_tensor(
                out=gatef[:], in0=gatef[:], in1=r16[:, :, None].to_broadcast(bc), op=AluOp.mult
            )
            nc.sync.dma_start(
                gate_out[:].rearrange("(p g) j -> p g j", g=G), gatef[:]
            )

            # ---- local top-1 over N per batch entry ----
            loc = bp.tile([BL, N], F32, tag="loc")
            iota_i = bp.tile([BL, N], I32, tag="iota_i")
            iota_f = bp.tile([BL, N], F32, tag="iota_f")
            lmax = bp.tile([BL, 1], F32, tag="lmax")
            lmask = bp.tile([BL, N], F32, tag="lmask")
            idxf = bp.tile([BL, 1], F32, tag="idxf")
            idx_i = bp.tile([BL, 1], I32, tag="idx_i")
            nc.sync.dma_start(loc_b[None, :], scoresT[E : E + 1, :])
            nc.sync.dma_start(loc[:], loc_b[:].rearrange("(b n) -> b n", n=N))
            nc.gpsimd.iota(iota_i[:], pattern=[[1, N]], base=0, channel_multiplier=N)
            nc.vector.tensor_copy(iota_f[:], iota_i[:])
            nc.vector.reduce_max(lmax[:], loc[:], axis=mybir.AxisListType.X)
            nc.vector.tensor_tensor(
                out=lmask[:], in0=loc[:], in1=lmax[:].to_broadcast([BL, N]), op=AluOp.is_ge
            )
            nc.vector.tensor_tensor(out=lmask[:], in0=lmask[:], in1=iota_f[:], op=AluOp.mult)
            nc.vector.reduce_max(idxf[:], lmask[:], axis=mybir.AxisListType.X)
            nc.vector.tensor_copy(idx_i[:], idxf[:])

            # ---- gather active tokens + their gates; AllGather ----
            agin = bp.tile([BL, D + E], F32, tag="agin")
            nc.gpsimd.indirect_dma_start(
                out=agin[:, 0:D],
                out_offset=None,
                in_=xrow[:],
                in_offset=IndirectOffsetOnAxis(ap=idx_i[:, :1], axis=0),
            )
            nc.gpsimd.indirect_dma_start(
                out=agin[:, D : D + E],
                out_offset=None,
                in_=gate_out[:],
                in_offset=IndirectOffsetOnAxis(ap=idx_i[:, :1], axis=0),
            )
            nc.sync.dma_start(ag_in[:], agin[:])
            nc.gpsimd.collective_compute(
                "AllGather", AluOp.bypass, replica_groups=groups,
                ins=[ag_in[:]], outs=[ag_out[:]],
            )

            # ---- Phase C: this core's expert over all 256 active tokens ----
            xa2 = bp.tile([128, MT, D + E], F32, tag="xa2")
            nc.sync.dma_start(xa2[:], ag_out[:].rearrange("(m p) c -> p m c", p=128))

            xaT = bp.tile([128, KD, A], F32R, tag="xaT")
            gallT = bp.tile([E, A], F32, tag="gallT")
            grow = bp.tile([128, A], F32, tag="grow")
            hsb = bp.tile([128, KH, A], F32R, tag="hsb")
            partial = bp.tile([128, KH, A], F32, tag="partial")
            ysb = bp.tile([128, MT, H], F32, tag="ysb")
            ymine = bp.tile([BL, H], F32, tag="ymine")

            with (
                tc.tile_pool(name="psC", bufs=2, space="PSUM") as psC,
                tc.tile_pool(name="psT2", bufs=4, space="PSUM") as psT2,
            ):
                for m in range(MT):
                    for k in range(KD):
                        tp = psT2.tile([128, 128], F32, tag="tr2")
                        nc.tensor.transpose(
                            tp[:], in_=xa2[:, m, k * 128 : (k + 1) * 128], identity=ident[:]
                        )
                        nc.vector.tensor_copy(xaT[:, k, m * 128 : (m + 1) * 128], tp[:])
                    gp = psT2.tile([128, 128], F32, tag="tr2")
                    nc.tensor.transpose(gp[0:E, :], in_=xa2[:, m, D : D + E], identity=ident[:])
                    nc.vector.tensor_copy(gallT[:, m * 128 : (m + 1) * 128], gp[0:E, :])

                gpp = psC.tile([128, A], F32, tag="mm")
                nc.tensor.matmul(gpp[:], lhsT=selsb[:], rhs=gallT[:], start=True, stop=True)
                nc.vector.tensor_copy(grow[:], gpp[:])

                # layer 1: hT = relu(W1.T @ x_activeT + b1)
                for m in range(KH):
                    hp = psC.tile([128, A], F32, tag="mm")
                    for k in range(KD):
                        nc.tensor.matmul(
                            hp[:],
                            lhsT=W1sb[:, k, m * 128 : (m + 1) * 128],
                            rhs=xaT[:, k, :],
                            start=(k == 0),
                            stop=(k == KD - 1),
                        )
                    nc.scalar.activation(
                        hsb[:, m, :], hp[:], Act.Relu, bias=b1csb[:, m : m + 1], scale=1.0
                    )
                # layer 2: partialT = g * (W2.T @ hT + b2)
                for m in range(KH):
                    op = psC.tile([128, A], F32, tag="mm")
                    for k in range(KH):
                        nc.tensor.matmul(
                            op[:],
                            lhsT=W2sb[:, k, m * 128 : (m + 1) * 128],
                            rhs=hsb[:, k, :],
                            start=(k == 0),
                            stop=(k == KH - 1),
                        )
                    nc.vector.scalar_tensor_tensor(
                        out=partial[:, m, :],
                        in0=op[:],
                        scalar=b2csb[:, m : m + 1],
                        in1=grow[:],
                        op0=AluOp.add,
                        op1=AluOp.mult,
                    )
                # transpose back to token-major; stream each 64KB block out to
                # the ReduceScatter input as soon as it is ready
                rs_v = rs_in[:].rearrange("(m p) h -> p m h", p=128)
                for m in range(KH):
                    for mt in range(MT):
                        tp = psT2.tile([128, 128], F32, tag="tr2")
                        nc.tensor.transpose(
                            tp[:], in_=partial[:, m, mt * 128 : (mt + 1) * 128], identity=ident[:]
                        )
                        nc.vector.tensor_copy(ysb[:, mt, m * 128 : (m + 1) * 128], tp[:])
                        nc.sync.dma_start(
                            rs_v[:, mt, m * 128 : (m + 1) * 128],
                            ysb[:, mt, m * 128 : (m + 1) * 128],
                        )
            nc.gpsimd.collective_compute(
                "ReduceScatter", AluOp.add, replica_groups=groups,
                ins=[rs_in[:]], outs=[rs_out[:]],
            )
            nc.sync.dma_start(ymine[:], rs_out[:])
            nc.gpsimd.indirect_dma_start(
                out=out[:],
                out_offset=IndirectOffsetOnAxis(ap=idx_i[:, :1], axis=0),
                in_=ymine[:],
                in_offset=None,
            )

    nc.compile()
    return nc


_NC = None


def _get_nc():
    global _NC
    if _NC is None:
        _NC = build()
    return _NC


def shard_inputs(x, W1, b1, W2, b2, gate_W, gate_b, lg_W, lg_b):
    f = lambda a: np.ascontiguousarray(np.asarray(a, dtype=np.float32))
    x = f(x).reshape(NCORES, TL, D)
    W1, b1, W2, b2 = f(W1), f(b1), f(W2), f(b2)
    gWc = np.ascontiguousarray(np.concatenate([f(gate_W), f(lg_W)], axis=1))
    gbc = np.ascontiguousarray(
        np.concatenate([f(gate_b), f(lg_b)])[:, None]
    )
    in_maps = []
    for c in range(NCORES):
        se = np.zeros((E, 128), np.float32)
        se[c, :] = 1.0
        in_maps.append(
            {
                "xT": np.ascontiguousarray(x[c].T),
                "xrow": x[c],
                "gW": gWc,
                "gb": gbc,
                "W1": W1[c],
                "W2": W2[c],
                "b1c": np.ascontiguousarray(b1[c].reshape(KH, 128).T),
                "b2c": np.ascontiguousarray(b2[c].reshape(KH, 128).T),
                "sel": se,
            }
        )
    return in_maps


def assemble(results):
    out = np.concatenate(
        [r["out"].reshape(BL, N, H) for r in results], axis=0
    )
    gate = np.concatenate(
        [r["gate_out"].reshape(BL, N, E) for r in results], axis=0
    )
    return out, gate


def run(in_maps, **kw):
    nc = _get_nc()
    return run_bass_kernel_spmd(nc, in_maps, core_ids=list(range(NCORES)), **kw)


def kernel(**inputs):
    in_maps = shard_inputs(**inputs)
    res = run(in_maps)
    return assemble(res.results)


# revision 12
# speedup vs baseline: 1.0805x; 1.0805x over previous
"""MoE routing kernel for 8 Trainium2 NeuronCores.

Problem: nn_MoE_hard (moe_routing). Reference computes, per token (B=256,N=64):
  gate_scores = renorm(top2mask(softmax(x @ gate_W + gate_b)))      [B,N,E]
  out = local_top1_mask(x@lg_W) * sum_e gate[e]*(relu(x@W1[e]+b1[e])@W2[e]+b2[e])
Only ACTIVE_K=1 of the N=64 rows per batch entry survives the local mask, so
only B=256 tokens need the expert MLP.  Strategy:
  Phase A (data-parallel over batch, 32 batch entries/core):
    gating softmax + top-2 renorm for all tokens (the gate_scores output),
    local top-1 over N, gather of each batch entry's single active token.
  Phase B: AllGather of the 256 active tokens (+ their gate rows).
  Phase C (expert-parallel, 1 expert/core): dense 2-layer MLP over all 256
    active tokens for this core's expert, gate-weighted; ReduceScatter sums
    expert contributions and hands each core its own 32 batch rows, which are
    scattered into the (zero-initialized) output.
"""

import numpy as np

import concourse.bass as bass
import concourse.mybir as mybir
import concourse.tile as tile
from concourse import bacc
from concourse.bass import IndirectOffsetOnAxis
from concourse.bass_utils import run_bass_kernel_spmd
from concourse.masks import make_identity
from concourse.tile_rust import add_dep_helper

F32 = mybir.dt.float32
F32R = mybir.dt.float32r
I32 = mybir.dt.int32

NCORES = 8
B, N, D = 256, 64, 1024
E, H = 8, 1024
BL = B // NCORES            # batch entries per core
TL = BL * N                 # tokens per core (2048)
G = TL // 128               # token groups of 128 (16)
J = E + 1                   # gate experts + local-gate column
A = B                       # total active tokens (ACTIVE_K=1 per batch entry)
KD = D // 128               # contraction chunks (8)
KH = H // 128               # hidden chunks (8)
MT = A // 128               # active-token 128-tiles (2)

AluOp = mybir.AluOpType
Act = mybir.ActivationFunctionType


def build():
    nc = bacc.Bacc("TRN2", num_devices=NCORES)

    xT = nc.dram_tensor("xT", [D, TL], F32, kind="ExternalInput")
    xrow = nc.dram_tensor("xrow", [TL, D], F32, kind="ExternalInput")
    gW = nc.dram_tensor("gW", [128, KD * J], F32, kind="ExternalInput")
    gb = nc.dram_tensor("gb", [J, 1], F32, kind="ExternalInput")
    W1 = nc.dram_tensor("W1", [D, H], F32R, kind="ExternalInput")
    W2 = nc.dram_tensor("W2", [H, H], F32R, kind="ExternalInput")
    b1c = nc.dram_tensor("b1c", [128, KH], F32, kind="ExternalInput")
    b2c = nc.dram_tensor("b2c", [128, KH], F32, kind="ExternalInput")
    sel = nc.dram_tensor("sel", [E, 128], F32, kind="ExternalInput")

    gate_out = nc.dram_tensor("gate_out", [TL, E], F32, kind="ExternalOutput")
    out = nc.dram_tensor("out", [TL, H], F32, kind="ExternalOutput")

    loc_b = nc.dram_tensor("loc_b", [TL], F32)
    ag_in = nc.dram_tensor("ag_in", [BL, D], F32)
    ag_out = nc.dram_tensor("ag_out", [A, D], F32, addr_space="Shared")
    ag2_in = nc.dram_tensor("ag2_in", [BL, E], F32)
    ag2_out = nc.dram_tensor("ag2_out", [A, E], F32, addr_space="Shared")
    rs_in = nc.dram_tensor("rs_in", [A, H], F32)
    rs_out = nc.dram_tensor("rs_out", [BL, H], F32)

    groups = [list(range(NCORES))]

    with tile.TileContext(nc, num_cores=NCORES) as tc:
        with (
            tc.tile_pool(name="const", bufs=1) as cp,
            tc.tile_pool(name="big", bufs=1) as bp,
        ):
            # ---- critical-path constants first (DMA queue priority) ----
            gWsb = cp.tile([128, KD, J], F32, tag="gWsb")
            nc.sync.dma_start(gWsb[:], gW[:].rearrange("p (k j) -> p k j", j=J))
            gbsb = cp.tile([J, 1], F32, tag="gbsb")
            nc.sync.dma_start(gbsb[:], gb[:])
            ident = cp.tile([128, 128], F32, tag="ident")
            make_identity(nc, ident[:])

            # ---- Phase A: gating for all local tokens ----
            # Pack the thin (M=9) gating matmul 4x across PE column groups:
            # k-chunk k accumulates into PSUM partitions [32*(k%4), 32*(k%4)+9).
            scoresT = bp.tile([J, TL], F32, tag="scoresT")
            scores3 = bp.tile([128, G, J], F32, tag="scores3")
            gatef = bp.tile([128, G, E], F32, tag="gatef")

            with (
                tc.tile_pool(name="xk", bufs=4) as xkp,
                tc.tile_pool(name="psA", bufs=1, space="PSUM") as psA,
                tc.tile_pool(name="psT", bufs=3, space="PSUM") as psT,
            ):
                sT_ps = psA.tile([128, TL], F32, tag="sT_ps")
                for k in range(KD):
                    xk = xkp.tile([128, TL], F32, tag="xk")
                    nc.sync.dma_start(xk[:], xT[k * 128 : (k + 1) * 128, :])
                    cg = 32 * (k % 4)
                    for n in range(TL // 512):
                        nc.tensor.matmul(
                            sT_ps[cg : cg + J, n * 512 : (n + 1) * 512],
                            lhsT=gWsb[:, k, :],
                            rhs=xk[:, n * 512 : (n + 1) * 512],
                            start=(k < 4),
                            stop=(k >= 4),
                            tile_position=(0, cg),
                        )
                # merge the 4 column groups + bias (PSUM -> SBUF); DVE may only
                # read one PSUM operand per op, so chain in-place adds
                nc.vector.tensor_scalar_add(scoresT[:], sT_ps[0:J, :], gbsb[:, 0:1])
                for cg in (32, 64, 96):
                    nc.vector.tensor_tensor(
                        out=scoresT[:], in0=scoresT[:], in1=sT_ps[cg : cg + J, :], op=AluOp.add
                    )

                # transpose to token-major [p, g, j]; token t = p*G + g
                sT_v = scoresT[:].rearrange("j (p g) -> j g p", g=G)
                for g in range(G):
                    tp = psT.tile([128, J], F32, tag="trA")
                    nc.tensor.transpose(tp[:], in_=sT_v[:, g, :], identity=ident[0:J, 0:J])
                    nc.vector.tensor_copy(scores3[:, g, :], tp[:])

            # ---- local top-1 over N per batch entry (before softmax: it only
            # needs scoresT row 8, and it gates the big x AllGather) ----
            loc = bp.tile([BL, N], F32, tag="loc")
            iota_i = bp.tile([BL, N], I32, tag="iota_i")
            iota_f = bp.tile([BL, N], F32, tag="iota_f")
            lmax = bp.tile([BL, 1], F32, tag="lmax")
            lmask = bp.tile([BL, N], F32, tag="lmask")
            idxf = bp.tile([BL, 1], F32, tag="idxf")
            idx_i = bp.tile([BL, 1], I32, tag="idx_i")
            nc.sync.dma_start(loc_b[None, :], scoresT[E : E + 1, :])
            nc.sync.dma_start(loc[:], loc_b[:].rearrange("(b n) -> b n", n=N))
            nc.gpsimd.iota(iota_i[:], pattern=[[1, N]], base=0, channel_multiplier=N)
            nc.vector.tensor_copy(iota_f[:], iota_i[:])
            nc.vector.reduce_max(lmax[:], loc[:], axis=mybir.AxisListType.X)
            nc.vector.tensor_tensor(
                out=lmask[:], in0=loc[:], in1=lmax[:].to_broadcast([BL, N]), op=AluOp.is_ge
            )
            nc.vector.tensor_tensor(out=lmask[:], in0=lmask[:], in1=iota_f[:], op=AluOp.mult)
            nc.vector.reduce_max(idxf[:], lmask[:], axis=mybir.AxisListType.X)
            nc.vector.tensor_copy(idx_i[:], idxf[:])

            # gather this core's 32 active token rows and AllGather them
            agin = bp.tile([BL, D], F32, tag="agin")
            nc.gpsimd.indirect_dma_start(
                out=agin[:],
                out_offset=None,
                in_=xrow[:],
                in_offset=IndirectOffsetOnAxis(ap=idx_i[:, :1], axis=0),
            )
            agin_dma = nc.sync.dma_start(ag_in[:], agin[:])
            nc.gpsimd.collective_compute(
                "AllGather", AluOp.bypass, replica_groups=groups,
                ins=[ag_in[:]], outs=[ag_out[:]],
            )

            # softmax over E (no max-subtract: logits are O(1)) + top-2 renorm
            expS = bp.tile([128, G, E], F32, tag="expS")
            probs = bp.tile([128, G, E], F32, tag="probs")
            m8 = bp.tile([128, G, E], F32, tag="m8")
            s16 = bp.tile([128, G], F32, tag="s16")
            r16 = bp.tile([128, G], F32, tag="r16")
            nc.scalar.activation(expS[:], scores3[:, :, 0:E], Act.Exp)
            nc.vector.reduce_sum(s16[:], expS[:], axis=mybir.AxisListType.X)
            nc.vector.reciprocal(r16[:], s16[:])
            bc = [128, G, E]
            nc.vector.tensor_tensor(
                out=probs[:], in0=expS[:], in1=r16[:, :, None].to_broadcast(bc), op=AluOp.mult
            )
            # top-1
            nc.vector.reduce_max(s16[:], probs[:], axis=mybir.AxisListType.X)
            nc.vector.tensor_tensor(
                out=m8[:], in0=probs[:], in1=s16[:, :, None].to_broadcast(bc), op=AluOp.is_ge
            )
            # knock out top-1 (probs <= 1, so subtracting the mask is enough)
            nc.vector.tensor_tensor(out=m8[:], in0=probs[:], in1=m8[:], op=AluOp.subtract)
            nc.vector.reduce_max(s16[:], m8[:], axis=mybir.AxisListType.X)
            # top-2 mask (>= second max)
            nc.vector.tensor_tensor(
                out=m8[:], in0=probs[:], in1=s16[:, :, None].to_broadcast(bc), op=AluOp.is_ge
            )
            nc.vector.tensor_tensor(out=gatef[:], in0=probs[:], in1=m8[:], op=AluOp.mult)
            nc.vector.reduce_sum(s16[:], gatef[:], axis=mybir.AxisListType.X)
            nc.vector.tensor_scalar_add(s16[:], s16[:], 1e-9)
            nc.vector.reciprocal(r16[:], s16[:])
            nc.vector.tensor_tensor(
                out=gatef[:], in0=gatef[:], in1=r16[:, :, None].to_broadcast(bc), op=AluOp.mult
            )
            nc.sync.dma_start(
                gate_out[:].rearrange("(p g) j -> p g j", g=G), gatef[:]
            )

            # ---- late bulk loads: start once the AG1 window opens ----
            selsb = cp.tile([E, 128], F32, tag="selsb")
            nc.sync.dma_start(selsb[:], sel[:])
            b1csb = cp.tile([128, KH], F32, tag="b1csb")
            nc.sync.dma_start(b1csb[:], b1c[:])
            b2csb = cp.tile([128, KH], F32, tag="b2csb")
            nc.sync.dma_start(b2csb[:], b2c[:])
            W1sb = bp.tile([128, KD, H], F32R, tag="W1sb")
            w1i = nc.scalar.dma_start(W1sb[:], W1[:].rearrange("(k p) h -> p k h", p=128))
            W2sb = bp.tile([128, KH, H], F32R, tag="W2sb")
            w2i = nc.scalar.dma_start(W2sb[:], W2[:].rearrange("(k p) h -> p k h", p=128))
            add_dep_helper(w1i.ins, agin_dma.ins, sync=True, reason="keep phase-A DMA bandwidth free")
            add_dep_helper(w2i.ins, agin_dma.ins, sync=True, reason="keep phase-A DMA bandwidth free")

            # ---- gather gate rows of active tokens; tiny second AllGather ----
            gact = bp.tile([BL, E], F32, tag="gact")
            nc.gpsimd.indirect_dma_start(
                out=gact[:],
                out_offset=None,
                in_=gate_out[:],
                in_offset=IndirectOffsetOnAxis(ap=idx_i[:, :1], axis=0),
            )
            nc.sync.dma_start(ag2_in[:], gact[:])
            nc.gpsimd.collective_compute(
                "AllGather", AluOp.bypass, replica_groups=groups,
                ins=[ag2_in[:]], outs=[ag2_out[:]],
            )

            # ---- Phase C: this core's expert over all 256 active tokens ----
            xa2 = bp.tile([128, MT, D], F32, tag="xa2")
            nc.sync.dma_start(xa2[:], ag_out[:].rearrange("(m p) c -> p m c", p=128))
            ga2 = bp.tile([128, MT, E], F32, tag="ga2")
            nc.sync.dma_start(ga2[:], ag2_out[:].rearrange("(m p) c -> p m c", p=128))

            xaT = bp.tile([128, KD, A], F32R, tag="xaT")
            gallT = bp.tile([E, A], F32, tag="gallT")
            grow = bp.tile([128, A], F32, tag="grow")
            hsb = bp.tile([128, KH, A], F32R, tag="hsb")
            partial = bp.tile([128, KH, A], F32, tag="partial")
            ysb = bp.tile([128, MT, H], F32, tag="ysb")
            ymine = bp.tile([BL, H], F32, tag="ymine")

            with (
                tc.tile_pool(name="psC", bufs=2, space="PSUM") as psC,
                tc.tile_pool(name="psT2", bufs=4, space="PSUM") as psT2,
            ):
                for m in range(MT):
                    for k in range(KD):
                        tp = psT2.tile([128, 128], F32, tag="tr2")
                        nc.tensor.transpose(
                            tp[:], in_=xa2[:, m, k * 128 : (k + 1) * 128], identity=ident[:]
                        )
                        nc.vector.tensor_copy(xaT[:, k, m * 128 : (m + 1) * 128], tp[:])
                    gp = psT2.tile([128, 128], F32, tag="tr2")
                    nc.tensor.transpose(gp[0:E, :], in_=ga2[:, m, :], identity=ident[:])
                    nc.vector.tensor_copy(gallT[:, m * 128 : (m + 1) * 128], gp[0:E, :])

                gpp = psC.tile([128, A], F32, tag="mm")
                nc.tensor.matmul(gpp[:], lhsT=selsb[:], rhs=gallT[:], start=True, stop=True)
                nc.vector.tensor_copy(grow[:], gpp[:])

                # layer 1: hT = relu(W1.T @ x_activeT + b1)
                for m in range(KH):
                    hp = psC.tile([128, A], F32, tag="mm")
                    for k in range(KD):
                        nc.tensor.matmul(
                            hp[:],
                            lhsT=W1sb[:, k, m * 128 : (m + 1) * 128],
                            rhs=xaT[:, k, :],
                            start=(k == 0),
                            stop=(k == KD - 1),
                        )
                    nc.scalar.activation(
                        hsb[:, m, :], hp[:], Act.Relu, bias=b1csb[:, m : m + 1], scale=1.0
                    )
                # layer 2: partialT = g * (W2.T @ hT + b2)
                for m in range(KH):
                    op = psC.tile([128, A], F32, tag="mm")
                    for k in range(KH):
                        nc.tensor.matmul(
                            op[:],
                            lhsT=W2sb[:, k, m * 128 : (m + 1) * 128],
                            rhs=hsb[:, k, :],
                            start=(k == 0),
                            stop=(k == KH - 1),
                        )
                    nc.vector.scalar_tensor_tensor(
                        out=partial[:, m, :],
                        in0=op[:],
                        scalar=b2csb[:, m : m + 1],
                        in1=grow[:],
                        op0=AluOp.add,
                        op1=AluOp.mult,
                    )
                # transpose back to token-major; stream each 64KB block out to
                # the ReduceScatter input as soon as it is ready
                rs_v = rs_in[:].rearrange("(m p) h -> p m h", p=128)
                for m in range(KH):
                    for mt in range(MT):
                        tp = psT2.tile([128, 128], F32, tag="tr2")
                        nc.tensor.transpose(
                            tp[:], in_=partial[:, m, mt * 128 : (mt + 1) * 128], identity=ident[:]
                        )
                        nc.vector.tensor_copy(ysb[:, mt, m * 128 : (m + 1) * 128], tp[:])
                        nc.sync.dma_start(
                            rs_v[:, mt, m * 128 : (m + 1) * 128],
                            ysb[:, mt, m * 128 : (m + 1) * 128],
                        )
            nc.gpsimd.collective_compute(
                "ReduceScatter", AluOp.add, replica_groups=groups,
                ins=[rs_in[:]], outs=[rs_out[:]],
            )
            nc.sync.dma_start(ymine[:], rs_out[:])
            nc.gpsimd.indirect_dma_start(
                out=out[:],
                out_offset=IndirectOffsetOnAxis(ap=idx_i[:, :1], axis=0),
                in_=ymine[:],
                in_offset=None,
            )

    nc.compile()
    return nc


_NC = None


def _get_nc():
    global _NC
    if _NC is None:
        _NC = build()
    return _NC


def shard_inputs(x, W1, b1, W2, b2, gate_W, gate_b, lg_W, lg_b):
    f = lambda a: np.ascontiguousarray(np.asarray(a, dtype=np.float32))
    x = f(x).reshape(NCORES, TL, D)
    W1, b1, W2, b2 = f(W1), f(b1), f(W2), f(b2)
    gWc = np.concatenate([f(gate_W), f(lg_W)], axis=1)          # [D, J]
    gWc = np.ascontiguousarray(
        gWc.reshape(KD, 128, J).transpose(1, 0, 2).reshape(128, KD * J)
    )
    gbc = np.ascontiguousarray(
        np.concatenate([f(gate_b), f(lg_b)])[:, None]
    )
    in_maps = []
    for c in range(NCORES):
        se = np.zeros((E, 128), np.float32)
        se[c, :] = 1.0
        in_maps.append(
            {
                "xT": np.ascontiguousarray(x[c].T),
                "xrow": x[c],
                "gW": gWc,
                "gb": gbc,
                "W1": W1[c],
                "W2": W2[c],
                "b1c": np.ascontiguousarray(b1[c].reshape(KH, 128).T),
                "b2c": np.ascontiguousarray(b2[c].reshape(KH, 128).T),
                "sel": se,
            }
        )
    return in_maps


def assemble(results):
    out = np.concatenate(
        [r["out"].reshape(BL, N, H) for r in results], axis=0
    )
    gate = np.concatenate(
        [r["gate_out"].reshape(BL, N, E) for r in results], axis=0
    )
    return out, gate


def run(in_maps, **kw):
    nc = _get_nc()
    return run_bass_kernel_spmd(nc, in_maps, core_ids=list(range(NCORES)), **kw)


def kernel(**inputs):
    in_maps = shard_inputs(**inputs)
    res = run(in_maps)
    return assemble(res.results)


# revision 13
# speedup vs baseline: 1.1347x; 1.0501x over previous
"""MoE routing kernel for 8 Trainium2 NeuronCores.

Problem: nn_MoE_hard (moe_routing). Reference computes, per token (B=256,N=64):
  gate_scores = renorm(top2mask(softmax(x @ gate_W + gate_b)))      [B,N,E]
  out = local_top1_mask(x@lg_W) * sum_e gate[e]*(relu(x@W1[e]+b1[e])@W2[e]+b2[e])
Only ACTIVE_K=1 of the N=64 rows per batch entry survives the local mask, so
only B=256 tokens need the expert MLP.  Strategy:
  Phase A (data-parallel over batch, 32 batch entries/core):
    gating softmax + top-2 renorm for all tokens (the gate_scores output),
    local top-1 over N, gather of each batch entry's single active token.
  Phase B: AllGather of the 256 active tokens (+ their gate rows).
  Phase C (expert-parallel, 1 expert/core): dense 2-layer MLP over all 256
    active tokens for this core's expert, gate-weighted; ReduceScatter sums
    expert contributions and hands each core its own 32 batch rows, which are
    scattered into the (zero-initialized) output.
"""

import numpy as np

import concourse.bass as bass
import concourse.mybir as mybir
import concourse.tile as tile
from concourse import bacc
from concourse.bass import IndirectOffsetOnAxis
from concourse.bass_utils import run_bass_kernel_spmd
from concourse.masks import make_identity
from concourse.tile_rust import add_dep_helper

F32 = mybir.dt.float32
F32R = mybir.dt.float32r
I32 = mybir.dt.int32

NCORES = 8
B, N, D = 256, 64, 1024
E, H = 8, 1024
BL = B // NCORES            # batch entries per core
TL = BL * N                 # tokens per core (2048)
G = TL // 128               # token groups of 128 (16)
J = E + 1                   # gate experts + local-gate column
A = B                       # total active tokens (ACTIVE_K=1 per batch entry)
KD = D // 128               # contraction chunks (8)
KH = H // 128               # hidden chunks (8)
MT = A // 128               # active-token 128-tiles (2)

AluOp = mybir.AluOpType
Act = mybir.ActivationFunctionType


def build():
    nc = bacc.Bacc("TRN2", num_devices=NCORES)

    xT = nc.dram_tensor("xT", [D, TL], F32, kind="ExternalInput")
    xrow = nc.dram_tensor("xrow", [TL, D], F32, kind="ExternalInput")
    gW = nc.dram_tensor("gW", [128, KD * J], F32, kind="ExternalInput")
    gb = nc.dram_tensor("gb", [J, 1], F32, kind="ExternalInput")
    W1 = nc.dram_tensor("W1", [D, H], F32R, kind="ExternalInput")
    W2 = nc.dram_tensor("W2", [H, H], F32R, kind="ExternalInput")
    b1c = nc.dram_tensor("b1c", [128, KH], F32, kind="ExternalInput")
    b2c = nc.dram_tensor("b2c", [128, KH], F32, kind="ExternalInput")
    sel = nc.dram_tensor("sel", [E, 128], F32, kind="ExternalInput")

    gate_out = nc.dram_tensor("gate_out", [TL, E], F32, kind="ExternalOutput")
    out = nc.dram_tensor("out", [TL, H], F32, kind="ExternalOutput")

    loc_b = nc.dram_tensor("loc_b", [TL], F32)
    bar_in = nc.dram_tensor("bar_in", [1, 8], F32)
    bar_out = nc.dram_tensor("bar_out", [NCORES, 8], F32, addr_space="Shared")
    ag_in = nc.dram_tensor("ag_in", [BL, D], F32)
    ag_out = nc.dram_tensor("ag_out", [A, D], F32, addr_space="Shared")
    ag2_in = nc.dram_tensor("ag2_in", [BL, E], F32)
    ag2_out = nc.dram_tensor("ag2_out", [A, E], F32, addr_space="Shared")
    rs_in = nc.dram_tensor("rs_in", [A, H], F32)
    rs_out = nc.dram_tensor("rs_out", [BL, H], F32)

    groups = [list(range(NCORES))]

    with tile.TileContext(nc, num_cores=NCORES) as tc:
        with (
            tc.tile_pool(name="const", bufs=1) as cp,
            tc.tile_pool(name="big", bufs=1) as bp,
        ):
            # ---- start-skew barrier: a tiny AllGather absorbs the cross-core
            # launch skew while the x load streams, so the real collectives
            # later don't each pay it ----
            barsb = cp.tile([1, 8], F32, tag="barsb")
            nc.gpsimd.memset(barsb[:], 0.0)
            nc.gpsimd.dma_start(bar_in[:], barsb[:])
            nc.gpsimd.collective_compute(
                "AllGather", AluOp.bypass, replica_groups=groups,
                ins=[bar_in[:]], outs=[bar_out[:]],
            )

            # ---- critical-path constants first (DMA queue priority) ----
            gWsb = cp.tile([128, KD, J], F32, tag="gWsb")
            nc.sync.dma_start(gWsb[:], gW[:].rearrange("p (k j) -> p k j", j=J))
            gbsb = cp.tile([J, 1], F32, tag="gbsb")
            nc.sync.dma_start(gbsb[:], gb[:])
            ident = cp.tile([128, 128], F32, tag="ident")
            make_identity(nc, ident[:])

            # ---- Phase A: gating for all local tokens ----
            # Pack the thin (M=9) gating matmul 4x across PE column groups:
            # k-chunk k accumulates into PSUM partitions [32*(k%4), 32*(k%4)+9).
            scoresT = bp.tile([J, TL], F32, tag="scoresT")
            scores3 = bp.tile([128, G, J], F32, tag="scores3")
            gatef = bp.tile([128, G, E], F32, tag="gatef")

            with (
                tc.tile_pool(name="xk", bufs=4) as xkp,
                tc.tile_pool(name="psA", bufs=1, space="PSUM") as psA,
                tc.tile_pool(name="psT", bufs=3, space="PSUM") as psT,
            ):
                sT_ps = psA.tile([128, TL], F32, tag="sT_ps")
                xk_dmas = []
                for k in range(KD):
                    xk = xkp.tile([128, TL], F32, tag="xk")
                    di = nc.sync.dma_start(xk[:], xT[k * 128 : (k + 1) * 128, :])
                    if k >= 2:
                        add_dep_helper(di.ins, xk_dmas[k - 2].ins, sync=True,
                                       reason="stagger x chunks for steady PE feed")
                    xk_dmas.append(di)
                    cg = 32 * (k % 4)
                    for n in range(TL // 512):
                        nc.tensor.matmul(
                            sT_ps[cg : cg + J, n * 512 : (n + 1) * 512],
                            lhsT=gWsb[:, k, :],
                            rhs=xk[:, n * 512 : (n + 1) * 512],
                            start=(k < 4),
                            stop=(k >= 4),
                            tile_position=(0, cg),
                        )
                # merge the 4 column groups + bias (PSUM -> SBUF); DVE may only
                # read one PSUM operand per op, so chain in-place adds
                nc.vector.tensor_scalar_add(scoresT[:], sT_ps[0:J, :], gbsb[:, 0:1])
                for cg in (32, 64, 96):
                    nc.vector.tensor_tensor(
                        out=scoresT[:], in0=scoresT[:], in1=sT_ps[cg : cg + J, :], op=AluOp.add
                    )

                # transpose to token-major [p, g, j]; token t = p*G + g
                sT_v = scoresT[:].rearrange("j (p g) -> j g p", g=G)
                for g in range(G):
                    tp = psT.tile([128, J], F32, tag="trA")
                    nc.tensor.transpose(tp[:], in_=sT_v[:, g, :], identity=ident[0:J, 0:J])
                    nc.vector.tensor_copy(scores3[:, g, :], tp[:])

            # ---- local top-1 over N per batch entry (before softmax: it only
            # needs scoresT row 8, and it gates the big x AllGather) ----
            loc = bp.tile([BL, N], F32, tag="loc")
            iota_i = bp.tile([BL, N], I32, tag="iota_i")
            iota_f = bp.tile([BL, N], F32, tag="iota_f")
            lmax = bp.tile([BL, 1], F32, tag="lmax")
            lmask = bp.tile([BL, N], F32, tag="lmask")
            idxf = bp.tile([BL, 1], F32, tag="idxf")
            idx_i = bp.tile([BL, 1], I32, tag="idx_i")
            nc.sync.dma_start(loc_b[None, :], scoresT[E : E + 1, :])
            nc.sync.dma_start(loc[:], loc_b[:].rearrange("(b n) -> b n", n=N))
            nc.gpsimd.iota(iota_i[:], pattern=[[1, N]], base=0, channel_multiplier=N)
            nc.vector.tensor_copy(iota_f[:], iota_i[:])
            nc.vector.reduce_max(lmax[:], loc[:], axis=mybir.AxisListType.X)
            nc.vector.tensor_tensor(
                out=lmask[:], in0=loc[:], in1=lmax[:].to_broadcast([BL, N]), op=AluOp.is_ge
            )
            nc.vector.tensor_tensor(out=lmask[:], in0=lmask[:], in1=iota_f[:], op=AluOp.mult)
            nc.vector.reduce_max(idxf[:], lmask[:], axis=mybir.AxisListType.X)
            nc.vector.tensor_copy(idx_i[:], idxf[:])

            # gather this core's 32 active token rows and AllGather them
            agin = bp.tile([BL, D], F32, tag="agin")
            nc.gpsimd.indirect_dma_start(
                out=agin[:],
                out_offset=None,
                in_=xrow[:],
                in_offset=IndirectOffsetOnAxis(ap=idx_i[:, :1], axis=0),
            )
            agin_dma = nc.sync.dma_start(ag_in[:], agin[:])
            nc.gpsimd.collective_compute(
                "AllGather", AluOp.bypass, replica_groups=groups,
                ins=[ag_in[:]], outs=[ag_out[:]],
            )

            # softmax over E (no max-subtract: logits are O(1)) + top-2 renorm
            expS = bp.tile([128, G, E], F32, tag="expS")
            probs = bp.tile([128, G, E], F32, tag="probs")
            m8 = bp.tile([128, G, E], F32, tag="m8")
            s16 = bp.tile([128, G], F32, tag="s16")
            r16 = bp.tile([128, G], F32, tag="r16")
            nc.scalar.activation(expS[:], scores3[:, :, 0:E], Act.Exp)
            nc.vector.reduce_sum(s16[:], expS[:], axis=mybir.AxisListType.X)
            nc.vector.reciprocal(r16[:], s16[:])
            bc = [128, G, E]
            nc.vector.tensor_tensor(
                out=probs[:], in0=expS[:], in1=r16[:, :, None].to_broadcast(bc), op=AluOp.mult
            )
            # top-1
            nc.vector.reduce_max(s16[:], probs[:], axis=mybir.AxisListType.X)
            nc.vector.tensor_tensor(
                out=m8[:], in0=probs[:], in1=s16[:, :, None].to_broadcast(bc), op=AluOp.is_ge
            )
            # knock out top-1 (probs <= 1, so subtracting the mask is enough)
            nc.vector.tensor_tensor(out=m8[:], in0=probs[:], in1=m8[:], op=AluOp.subtract)
            nc.vector.reduce_max(s16[:], m8[:], axis=mybir.AxisListType.X)
            # top-2 mask (>= second max)
            nc.vector.tensor_tensor(
                out=m8[:], in0=probs[:], in1=s16[:, :, None].to_broadcast(bc), op=AluOp.is_ge
            )
            nc.vector.tensor_tensor(out=gatef[:], in0=probs[:], in1=m8[:], op=AluOp.mult)
            nc.vector.reduce_sum(s16[:], gatef[:], axis=mybir.AxisListType.X)
            nc.vector.tensor_scalar_add(s16[:], s16[:], 1e-9)
            nc.vector.reciprocal(r16[:], s16[:])
            nc.vector.tensor_tensor(
                out=gatef[:], in0=gatef[:], in1=r16[:, :, None].to_broadcast(bc), op=AluOp.mult
            )
            nc.sync.dma_start(
                gate_out[:].rearrange("(p g) j -> p g j", g=G), gatef[:]
            )

            # ---- late bulk loads: start once the AG1 window opens ----
            selsb = cp.tile([E, 128], F32, tag="selsb")
            nc.sync.dma_start(selsb[:], sel[:])
            b1csb = cp.tile([128, KH], F32, tag="b1csb")
            nc.sync.dma_start(b1csb[:], b1c[:])
            b2csb = cp.tile([128, KH], F32, tag="b2csb")
            nc.sync.dma_start(b2csb[:], b2c[:])
            W1sb = bp.tile([128, KD, H], F32R, tag="W1sb")
            w1i = nc.scalar.dma_start(W1sb[:], W1[:].rearrange("(k p) h -> p k h", p=128))
            W2sb = bp.tile([128, KH, H], F32R, tag="W2sb")
            w2i = nc.scalar.dma_start(W2sb[:], W2[:].rearrange("(k p) h -> p k h", p=128))
            add_dep_helper(w1i.ins, agin_dma.ins, sync=True, reason="keep phase-A DMA bandwidth free")
            add_dep_helper(w2i.ins, agin_dma.ins, sync=True, reason="keep phase-A DMA bandwidth free")

            # ---- gather gate rows of active tokens; tiny second AllGather ----
            gact = bp.tile([BL, E], F32, tag="gact")
            nc.gpsimd.indirect_dma_start(
                out=gact[:],
                out_offset=None,
                in_=gate_out[:],
                in_offset=IndirectOffsetOnAxis(ap=idx_i[:, :1], axis=0),
            )
            nc.sync.dma_start(ag2_in[:], gact[:])
            nc.gpsimd.collective_compute(
                "AllGather", AluOp.bypass, replica_groups=groups,
                ins=[ag2_in[:]], outs=[ag2_out[:]],
            )

            # ---- Phase C: this core's expert over all 256 active tokens ----
            xa2 = bp.tile([128, MT, D], F32, tag="xa2")
            nc.sync.dma_start(xa2[:], ag_out[:].rearrange("(m p) c -> p m c", p=128))
            ga2 = bp.tile([128, MT, E], F32, tag="ga2")
            nc.sync.dma_start(ga2[:], ag2_out[:].rearrange("(m p) c -> p m c", p=128))

            xaT = bp.tile([128, KD, A], F32R, tag="xaT")
            gallT = bp.tile([E, A], F32, tag="gallT")
            grow = bp.tile([128, A], F32, tag="grow")
            hsb = bp.tile([128, KH, A], F32R, tag="hsb")
            partial = bp.tile([128, KH, A], F32, tag="partial")
            ysb = bp.tile([128, MT, H], F32, tag="ysb")
            ymine = bp.tile([BL, H], F32, tag="ymine")

            with (
                tc.tile_pool(name="psC", bufs=2, space="PSUM") as psC,
                tc.tile_pool(name="psT2", bufs=4, space="PSUM") as psT2,
            ):
                for m in range(MT):
                    for k in range(KD):
                        tp = psT2.tile([128, 128], F32, tag="tr2")
                        nc.tensor.transpose(
                            tp[:], in_=xa2[:, m, k * 128 : (k + 1) * 128], identity=ident[:]
                        )
                        nc.vector.tensor_copy(xaT[:, k, m * 128 : (m + 1) * 128], tp[:])
                    gp = psT2.tile([128, 128], F32, tag="tr2")
                    nc.tensor.transpose(gp[0:E, :], in_=ga2[:, m, :], identity=ident[:])
                    nc.vector.tensor_copy(gallT[:, m * 128 : (m + 1) * 128], gp[0:E, :])

                gpp = psC.tile([128, A], F32, tag="mm")
                nc.tensor.matmul(gpp[:], lhsT=selsb[:], rhs=gallT[:], start=True, stop=True)
                nc.vector.tensor_copy(grow[:], gpp[:])

                # layer 1: hT = relu(W1.T @ x_activeT + b1)
                for m in range(KH):
                    hp = psC.tile([128, A], F32, tag="mm")
                    for k in range(KD):
                        nc.tensor.matmul(
                            hp[:],
                            lhsT=W1sb[:, k, m * 128 : (m + 1) * 128],
                            rhs=xaT[:, k, :],
                            start=(k == 0),
                            stop=(k == KD - 1),
                        )
                    nc.scalar.activation(
                        hsb[:, m, :], hp[:], Act.Relu, bias=b1csb[:, m : m + 1], scale=1.0
                    )
                # layer 2: partialT = g * (W2.T @ hT + b2)
                for m in range(KH):
                    op = psC.tile([128, A], F32, tag="mm")
                    for k in range(KH):
                        nc.tensor.matmul(
                            op[:],
                            lhsT=W2sb[:, k, m * 128 : (m + 1) * 128],
                            rhs=hsb[:, k, :],
                            start=(k == 0),
                            stop=(k == KH - 1),
                        )
                    nc.vector.scalar_tensor_tensor(
                        out=partial[:, m, :],
                        in0=op[:],
                        scalar=b2csb[:, m : m + 1],
                        in1=grow[:],
                        op0=AluOp.add,
                        op1=AluOp.mult,
                    )
                # transpose back to token-major; stream each 64KB block out to
                # the ReduceScatter input as soon as it is ready
                rs_v = rs_in[:].rearrange("(m p) h -> p m h", p=128)
                for m in range(KH):
                    for mt in range(MT):
                        tp = psT2.tile([128, 128], F32, tag="tr2")
                        nc.tensor.transpose(
                            tp[:], in_=partial[:, m, mt * 128 : (mt + 1) * 128], identity=ident[:]
                        )
                        nc.vector.tensor_copy(ysb[:, mt, m * 128 : (m + 1) * 128], tp[:])
                        nc.sync.dma_start(
                            rs_v[:, mt, m * 128 : (m + 1) * 128],
                            ysb[:, mt, m * 128 : (m + 1) * 128],
                        )
            nc.gpsimd.collective_compute(
                "ReduceScatter", AluOp.add, replica_groups=groups,
                ins=[rs_in[:]], outs=[rs_out[:]],
            )
            nc.sync.dma_start(ymine[:], rs_out[:])
            nc.gpsimd.indirect_dma_start(
                out=out[:],
                out_offset=IndirectOffsetOnAxis(ap=idx_i[:, :1], axis=0),
                in_=ymine[:],
                in_offset=None,
            )

    nc.compile()
    return nc


_NC = None


def _get_nc():
    global _NC
    if _NC is None:
        _NC = build()
    return _NC


def shard_inputs(x, W1, b1, W2, b2, gate_W, gate_b, lg_W, lg_b):
    f = lambda a: np.ascontiguousarray(np.asarray(a, dtype=np.float32))
    x = f(x).reshape(NCORES, TL, D)
    W1, b1, W2, b2 = f(W1), f(b1), f(W2), f(b2)
    gWc = np.concatenate([f(gate_W), f(lg_W)], axis=1)          # [D, J]
    gWc = np.ascontiguousarray(
        gWc.reshape(KD, 128, J).transpose(1, 0, 2).reshape(128, KD * J)
    )
    gbc = np.ascontiguousarray(
        np.concatenate([f(gate_b), f(lg_b)])[:, None]
    )
    in_maps = []
    for c in range(NCORES):
        se = np.zeros((E, 128), np.float32)
        se[c, :] = 1.0
        in_maps.append(
            {
                "xT": np.ascontiguousarray(x[c].T),
                "xrow": x[c],
                "gW": gWc,
                "gb": gbc,
                "W1": W1[c],
                "W2": W2[c],
                "b1c": np.ascontiguousarray(b1[c].reshape(KH, 128).T),
                "b2c": np.ascontiguousarray(b2[c].reshape(KH, 128).T),
                "sel": se,
            }
        )
    return in_maps


def assemble(results):
    out = np.concatenate(
        [r["out"].reshape(BL, N, H) for r in results], axis=0
    )
    gate = np.concatenate(
        [r["gate_out"].reshape(BL, N, E) for r in results], axis=0
    )
    return out, gate


def run(in_maps, **kw):
    nc = _get_nc()
    return run_bass_kernel_spmd(nc, in_maps, core_ids=list(range(NCORES)), **kw)


def kernel(**inputs):
    in_maps = shard_inputs(**inputs)
    res = run(in_maps)
    return assemble(res.results)
